# revision 1
# baseline (speedup 1.0000x reference)
"""Trainium2 Bass kernel for nn_BDHTinyModel (4-layer weight-shared tiny transformer).

Sharding: 8 NeuronCores = 4 batch groups x 2 tensor-parallel halves.
Core c handles batch b=c//2 and heads [4*(c%2), 4*(c%2)+4). All four heads'
y@encoder partials accumulate into one vd buffer; each t-half is pair
all-reduced as soon as it completes (2 collectives/layer). Each core
computes logits for half the vocab (written bf16, upcast on host).

Key tricks:
- activations kept feature-on-partition ([n, t] / [d, t]); rope made
  lane-local by de-interleaving even/odd channels host-side (exact: the
  n-contraction is order-invariant).
- qr@qr scores run in fp8e4 DoubleRow (2 n-chunks contracted per MM, ~2x);
  residual stream v is bf16. Measured rel-l2 vs f32 reference ~1.04e-2.
- the whole layer is emitted in explicit pipeline order at t-half (c)
  granularity: engine queues are FIFO, so emission order is per-engine
  execution order. At each layer boundary: update-half-A -> next-layer
  c0 work -> update-half-B -> c1 work, and head-2's c1 tail back-half is
  deferred past head-3's c0 so the first collective is fully covered.
- PSUM tags split by role (mm/xp1/tp/stat) so groups that wait on the
  collective never clog the main matmul ring.
"""

import math

import numpy as np
import ml_dtypes

import concourse.bass as bass
import concourse.mybir as mybir
import concourse.tile as tile
from concourse import bacc
from concourse.bass_utils import run_bass_kernel_spmd
from concourse.masks import make_identity

# model dims (hardcoded per the problem spec)
B, T, D, NH, N, VOCAB, NL = 4, 1024, 512, 8, 1024, 32000, 4
EPS = 1e-5
P = 128
NHL = NH // 2          # heads per core
VLOC = VOCAB // 2      # vocab half per core
N_CORES = 8
GROUPS = [[0, 1], [2, 3], [4, 5], [6, 7]]

f32 = mybir.dt.float32
bf16 = mybir.dt.bfloat16
f8e4 = mybir.dt.float8e4
i32 = mybir.dt.int32
FP8_SCORES = True  # qr@qr in fp8e4 DoubleRow (2 n-chunks contracted per MM)
Alu = mybir.AluOpType
Act = mybir.ActivationFunctionType
AX = mybir.AxisListType


def build_nc(reps=1, no_cc=False):
    nc = bacc.Bacc(num_devices=N_CORES)

    EMB = nc.declare_dram_parameter("emb", [VOCAB, D], f32, isOutput=False)
    IDX = nc.declare_dram_parameter("idx", [P, T // P], i32, isOutput=False)
    WX = nc.declare_dram_parameter("wx", [P, NHL, 8, 4, P], bf16, isOutput=False)
    WY = nc.declare_dram_parameter("wy", [P, NHL, 8, 4, P], bf16, isOutput=False)
    ENC = nc.declare_dram_parameter("enc", [P, NHL, 8, D], bf16, isOutput=False)
    LMH = nc.declare_dram_parameter("lmh", [P, 4, VLOC], bf16, isOutput=False)
    COS = nc.declare_dram_parameter("cos", [P, 4, T], bf16, isOutput=False)
    SIN = nc.declare_dram_parameter("sin", [P, 4, T], bf16, isOutput=False)
    MASK = nc.declare_dram_parameter("mask", [P, 512], bf16, isOutput=False)
    OUT = nc.declare_dram_parameter("out", [T, VLOC], bf16, isOutput=True)

    with tile.TileContext(nc) as tc:
        with (
            tc.tile_pool(name="wpool", bufs=1) as wpool,
            tc.tile_pool(name="vpool", bufs=1) as vpool,
            tc.tile_pool(name="spool", bufs=4) as spool,
            tc.tile_pool(name="mmps", bufs=3, space="PSUM") as mmps,
            tc.tile_pool(name="stps", bufs=1, space="PSUM") as stps,
            tc.tile_pool(name="drp", bufs=2, space="DRAM") as drp,
        ):
            # ---- resident constants/weights ----
            # idx first: the embedding gathers wait on it, and the big
            # cos/sin transfers would otherwise block it in the SP FIFO
            idx_sb = wpool.tile([P, T // P], i32)
            nc.sync.dma_start(idx_sb[:], IDX[:])
            mask_sb = wpool.tile([P, 512], bf16)
            nc.sync.dma_start(mask_sb[:], MASK[:])
            cos_sb = wpool.tile([P, 4, T], bf16)
            nc.sync.dma_start(cos_sb[:], COS[:])
            sin_sb = wpool.tile([P, 4, T], bf16)
            nc.sync.dma_start(sin_sb[:], SIN[:])
            ident = wpool.tile([P, P], bf16)
            make_identity(nc, ident[:])
            ones = wpool.tile([P, 1], bf16)
            nc.vector.memset(ones[:], 1.0)
            epsb = wpool.tile([P, 1], f32)
            nc.vector.memset(epsb[:], EPS)

            # ---- v state (bf16 residual stream; rel-err budget covers it) ----
            vbf = vpool.tile([P, 8, D], bf16)   # v in [t, d] layout
            vT = vpool.tile([P, 4, T], bf16)    # v^T in [d, t] layout

            def transpose_tt(tt):
                # vbf[t,d] tt-chunk -> vT [d,t], 4 PE transposes of 128x128
                # own PSUM tag: must not block the mm ring while waiting on
                # the vd AllReduce
                for ds in range(4):
                    ps = mmps.tile([P, P], bf16, tag="tp", bufs=2, name="tp_ps")
                    nc.tensor.transpose(
                        ps[:], vbf[:, tt, ds * P:(ds + 1) * P], ident[:]
                    )
                    nc.vector.tensor_copy(vT[:, ds, tt * P:(tt + 1) * P], ps[:])

            for rep in range(reps):
                # ---- embedding gather + LayerNorm -> v0 (pipelined per tt) ----
                with tc.tile_pool(name=f"gpool{rep}", bufs=3) as gpool:
                    for tt in range(8):
                        g = gpool.tile([P, D], f32, tag="gather")
                        nc.gpsimd.indirect_dma_start(
                            out=g[:],
                            out_offset=None,
                            in_=EMB[:],
                            in_offset=bass.IndirectOffsetOnAxis(
                                ap=idx_sb[:, tt:tt + 1], axis=0
                            ),
                        )
                        s = spool.tile([P, 1], f32, tag="s")
                        nc.vector.tensor_reduce(s[:], g[:], axis=AX.X, op=Alu.add)
                        q = spool.tile([P, 1], f32, tag="q")
                        dummy = gpool.tile([P, D], bf16, tag="sqd")
                        nc.scalar.activation(dummy[:], g[:], Act.Square, accum_out=q[:])
                        mu = spool.tile([P, 1], f32, tag="mu")
                        nc.scalar.mul(mu[:], s[:], 1.0 / D)
                        mu2 = spool.tile([P, 1], f32, tag="mu2")
                        nc.vector.tensor_tensor(mu2[:], mu[:], mu[:], op=Alu.mult)
                        var = spool.tile([P, 1], f32, tag="var")
                        nc.vector.scalar_tensor_tensor(
                            var[:], q[:], 1.0 / D, mu2[:], Alu.mult, Alu.subtract
                        )
                        std = spool.tile([P, 1], f32, tag="std")
                        nc.scalar.activation(std[:], var[:], Act.Sqrt, bias=epsb[:])
                        rsq = spool.tile([P, 1], f32, tag="rsq")
                        nc.vector.reciprocal(rsq[:], std[:])
                        nc.vector.tensor_scalar(
                            vbf[:, tt, :], g[:], mu[:], rsq[:], Alu.subtract, Alu.mult
                        )
                        transpose_tt(tt)

                # ---- layers + lm head, explicitly software-pipelined ----
                # Engine queues are FIFO, so emission order ~= per-engine
                # execution order. At each layer boundary we emit:
                #   update-half-A -> next-layer c0 work -> update-half-B -> c1
                # so the PE always has chunk-A work while the second half of
                # the vd AllReduce is in flight.
                with (
                    tc.tile_pool(name=f"slab{rep}", bufs=4) as slab,
                    tc.tile_pool(name=f"atp{rep}", bufs=2) as atp,
                    tc.tile_pool(name=f"scrp{rep}", bufs=1) as scrp,
                    tc.tile_pool(name=f"sqp{rep}", bufs=4) as sqp,
                    tc.tile_pool(name=f"encp{rep}", bufs=2) as encp,
                    tc.tile_pool(name=f"wxp{rep}", bufs=2) as wxp,
                    tc.tile_pool(name=f"wyp{rep}", bufs=2) as wyp,
                    tc.tile_pool(name=f"lmhp{rep}", bufs=2) as lmhp,
                    tc.tile_pool(name=f"obp{rep}", bufs=3) as obp,
                ):
                    XW = {}   # (layer, h) -> (XT, wxh)
                    TS = {}   # (layer, h) -> tail tiles
                    ATS = {}  # (layer, h, c) -> a^T half-tile
                    vds = {}  # layer -> vd accumulator

                    def get_vd(layer):
                        if layer not in vds:
                            vds[layer] = vpool.tile(
                                [P, 8, D], bf16,
                                tag="vda" if layer % 2 == 0 else "vdb",
                                name=f"vd_{rep}_{layer}",
                            )
                        return vds[layer]

                    def X(layer, h, c, strips=False):
                        if (layer, h) not in XW:
                            XT = slab.tile([P, 8, T], bf16, tag="slab", name=f"xt_{rep}_{layer}_{h}")
                            wxh = wxp.tile([P, 8, 4, P], bf16, tag="wx", name=f"wx_{rep}_{layer}_{h}")
                            nc.sync.dma_start(wxh[:], WX[:, h])
                            XW[(layer, h)] = (XT, wxh)
                        XT, wxh = XW[(layer, h)]
                        if strips:
                            # boundary variant: 128-col strips start as soon as
                            # each tt-chunk of the updated v^T lands
                            for st_ in range(4):
                                t0 = c * 512 + st_ * 128
                                for nt in range(8):
                                    ps = mmps.tile([P, 128], f32, tag="mm", bufs=3, name="x_ps")
                                    for ds in range(4):
                                        nc.tensor.matmul(
                                            ps[:],
                                            wxh[:, nt, ds, :],
                                            vT[:, ds, t0:t0 + 128],
                                            start=(ds == 0),
                                            stop=(ds == 3),
                                        )
                                    nc.scalar.activation(
                                        XT[:, nt, t0:t0 + 128], ps[:], Act.Relu
                                    )
                            return
                        for nt in range(8):
                            ps = mmps.tile([P, 512], f32, tag="mm" if c == 0 else "xp1",
                                           bufs=3 if c == 0 else 2, name="x_ps")
                            for ds in range(4):
                                nc.tensor.matmul(
                                    ps[:],
                                    wxh[:, nt, ds, :],
                                    vT[:, ds, c * 512:(c + 1) * 512],
                                    start=(ds == 0),
                                    stop=(ds == 3),
                                )
                            nc.scalar.activation(
                                XT[:, nt, c * 512:(c + 1) * 512], ps[:], Act.Relu
                            )

                    def emit_ar(vd, layer, half):
                        # pair AllReduce of a t-chunk of the vd partial; all
                        # traffic rides the Pool queue so the SP prefetch
                        # stream is never blocked behind a collective wait
                        cc_in = drp.tile([T // 2, D], bf16, tag="cc_in", name=f"cc_in_{rep}_{layer}_{half}")
                        cc_out = drp.tile([T // 2, D], bf16, tag="cc_out", name=f"cc_out_{rep}_{layer}_{half}")
                        nc.gpsimd.dma_start(
                            cc_in.rearrange("(o p) d -> p o d", p=P),
                            vd[:, 4 * half:4 * half + 4, :],
                        )
                        nc.gpsimd.collective_compute(
                            "AllReduce", Alu.add,
                            replica_groups=GROUPS,
                            ins=[cc_in[:]],
                            outs=[cc_out[:]],
                        )
                        for pr in range(2):
                            nc.gpsimd.dma_start(
                                vd[:, 4 * half + 2 * pr:4 * half + 2 * pr + 2, :],
                                cc_out[256 * pr:256 * pr + 256, :]
                                .rearrange("(o p) d -> p o d", p=P),
                            )

                    def S(layer, h, c, part="all"):
                        if (layer, h) not in TS:
                            QR = slab.tile([P, 8, T], f8e4 if FP8_SCORES else bf16,
                                           tag="slab", name=f"qr_{rep}_{layer}_{h}")
                            ST = slab.tile([P, 8, T], bf16, tag="slab", name=f"st_{rep}_{layer}_{h}")
                            rsqh = spool.tile([P, 8], f32, tag="rsqh", name=f"rsqh_{rep}_{layer}_{h}")
                            wyh = wyp.tile([P, 8, 4, P], bf16, tag="wy", name=f"wy_{rep}_{layer}_{h}")
                            nc.sync.dma_start(wyh[:], WY[:, h])
                            ench = []
                            for ns in range(8):
                                e = encp.tile([P, D], bf16, tag=f"ench{h % 2}", bufs=8,
                                              name=f"ench_{rep}_{layer}_{h}_{ns}")
                                nc.sync.dma_start(e[:], ENC[:, h, ns])
                                ench.append(e)
                            TS[(layer, h)] = (QR, ST, rsqh, wyh, ench)
                        QR, ST, rsqh, wyh, ench = TS[(layer, h)]
                        XT = XW[(layer, h)][0]
                        vd = get_vd(layer)
                        cs = slice(c * 512, (c + 1) * 512)

                        if part == "back":
                            emit_tail_back(layer, h, c, XT, ATS.pop((layer, h, c)), rsqh, wyh, ench, vd)
                            return
                        # rope; bf16 intermediates, single final rounding into QR
                        tA = scrp.tile([P, 4, 512], bf16, tag="scr", bufs=2, name="rope_tA")
                        nc.vector.tensor_tensor(tA[:], XT[:, 0:4, cs], cos_sb[:, :, cs], op=Alu.mult)
                        tB = scrp.tile([P, 4, 512], bf16, tag="scr", bufs=2, name="rope_tB")
                        nc.vector.tensor_tensor(tB[:], XT[:, 4:8, cs], sin_sb[:, :, cs], op=Alu.mult)
                        nc.vector.tensor_tensor(QR[:, 0:4, cs], tA[:], tB[:], op=Alu.subtract)
                        tC = scrp.tile([P, 4, 512], bf16, tag="scr", bufs=2, name="rope_tC")
                        nc.vector.tensor_tensor(tC[:], XT[:, 0:4, cs], sin_sb[:, :, cs], op=Alu.mult)
                        tD = scrp.tile([P, 4, 512], bf16, tag="scr", bufs=2, name="rope_tD")
                        nc.vector.tensor_tensor(tD[:], XT[:, 4:8, cs], cos_sb[:, :, cs], op=Alu.mult)
                        nc.vector.tensor_tensor(QR[:, 4:8, cs], tC[:], tD[:], op=Alu.add)

                        # scores^T (strictly causal s<t), symmetric trick;
                        # fp8 DoubleRow contracts ns-chunk pairs in one MM
                        for k in range(8):
                            if k * 128 >= (c + 1) * 512:
                                continue
                            if (k + 1) * 128 <= c * 512:
                                t0, w, diag = c * 512, 512, False
                            else:
                                t0, w, diag = k * 128, 512 * (c + 1) - k * 128, True
                            ps = mmps.tile([P, 512], f32, tag="mm", name="s_ps")
                            if FP8_SCORES:
                                for i in range(4):
                                    nc.tensor.matmul(
                                        ps[:, :w],
                                        QR[:, 2 * i:2 * i + 2, k * 128:(k + 1) * 128],
                                        QR[:, 2 * i:2 * i + 2, t0:t0 + w],
                                        start=(i == 0),
                                        stop=(i == 3),
                                        perf_mode=mybir.MatmulPerfMode.DoubleRow,
                                    )
                            else:
                                for ns in range(8):
                                    nc.tensor.matmul(
                                        ps[:, :w],
                                        QR[:, ns, k * 128:(k + 1) * 128],
                                        QR[:, ns, t0:t0 + w],
                                        start=(ns == 0),
                                        stop=(ns == 7),
                                    )
                            if not diag:
                                nc.scalar.copy(ST[:, k, t0:t0 + w], ps[:, :w])
                            else:
                                # mask[p, j] = (p < j) is all-ones for j >= 128:
                                # only the leading diagonal block needs the
                                # multiply; the rest is a plain (ACT) copy
                                nc.vector.tensor_tensor(
                                    ST[:, k, t0:t0 + 128],
                                    ps[:, :128],
                                    mask_sb[:, :128],
                                    op=Alu.mult,
                                )
                                if w > 128:
                                    nc.scalar.copy(
                                        ST[:, k, t0 + 128:t0 + w], ps[:, 128:w]
                                    )

                        # a^T = v^T @ S_masked ([d, t] layout) + rms stats
                        AT = atp.tile([P, 4, 512], bf16, tag="at", bufs=3, name=f"at_{rep}_{layer}_{h}_{c}")
                        sqs = []
                        for dt in range(4):
                            ps = mmps.tile([P, 512], f32, tag="mm", name="a_ps")
                            nss = range(4) if c == 0 else range(8)
                            for i, ns in enumerate(nss):
                                # ST[:, ns, *] is zero for t <= 128*ns: shrink
                                w0 = max(ns * 128 - c * 512, 0)
                                nc.tensor.matmul(
                                    ps[:, w0:512],
                                    vbf[:, ns, dt * 128:(dt + 1) * 128],
                                    ST[:, ns, c * 512 + w0:(c + 1) * 512],
                                    start=(i == 0),
                                    stop=(i == len(nss) - 1),
                                )
                            nc.scalar.copy(AT[:, dt, :], ps[:])
                            sq = sqp.tile([P, 512], bf16, tag="sq", bufs=2, name="sq")
                            nc.vector.tensor_tensor(sq[:], AT[:, dt, :], AT[:, dt, :], op=Alu.mult)
                            sqs.append(sq)
                        sps = stps.tile([1, 512], f32, tag="stat", name="a_stat")
                        for dt in range(4):
                            nc.tensor.matmul(
                                sps[:],
                                ones[:],
                                sqs[dt][:],
                                start=(dt == 0),
                                stop=(dt == 3),
                            )
                        rsq_row = spool.tile([1, 512], f32, tag="rsqrow", bufs=1, name=f"rsqrow_{rep}_{layer}_{h}_{c}")
                        nc.scalar.activation(
                            rsq_row[:], sps[:], Act.Sqrt,
                            bias=epsb[:1], scale=1.0 / D,
                        )
                        nc.vector.reciprocal(rsq_row[:], rsq_row[:])
                        # transpose [1, 512] -> [t-partition, 4] via DRAM round-trip
                        rdr = drp.tile([512], f32, tag="rsqdr", name=f"rsqdr_{rep}_{layer}_{h}_{c}")
                        nc.sync.dma_start(rdr[:], rsq_row[0:1, :])
                        nc.sync.dma_start(
                            rsqh[:, 4 * c:4 * c + 4],
                            rdr.rearrange("(o p) -> p o", p=P),
                        )

                        ATS[(layer, h, c)] = AT
                        if part == "front":
                            return
                        emit_tail_back(layer, h, c, XT, ATS.pop((layer, h, c)), rsqh, wyh, ench, vd)

                    def emit_tail_back(layer, h, c, XT, AT, rsqh, wyh, ench, vd):
                        cs = slice(c * 512, (c + 1) * 512)
                        # y = relu(ln(a) @ Wy) * x, written in place into XT
                        for nt in range(8):
                            ps = mmps.tile([P, 512], f32, tag="mm", name="z_ps")
                            for ds in range(4):
                                nc.tensor.matmul(
                                    ps[:],
                                    wyh[:, nt, ds, :],
                                    AT[:, ds, :],
                                    start=(ds == 0),
                                    stop=(ds == 3),
                                )
                            zz = sqp.tile([P, 512], bf16, tag="zz", bufs=2, name="zz")
                            nc.scalar.activation(zz[:], ps[:], Act.Relu)
                            nc.vector.tensor_tensor(
                                XT[:, nt, cs], zz[:], XT[:, nt, cs], op=Alu.mult
                            )

                        # vd partial: (y^T)^T @ enc, scaled by rsq_h at evac
                        for tt in range(4 * c, 4 * c + 4):
                            ps = mmps.tile([P, 512], f32, tag="mm", name="vd_ps")
                            for ns in range(8):
                                nc.tensor.matmul(
                                    ps[:],
                                    XT[:, ns, tt * 128:(tt + 1) * 128],
                                    ench[ns][:],
                                    start=(ns == 0),
                                    stop=(ns == 7),
                                )
                            if h == 0:
                                nc.scalar.activation(
                                    vd[:, tt, :], ps[:], Act.Copy, scale=rsqh[:, tt:tt + 1]
                                )
                            else:
                                nc.vector.scalar_tensor_tensor(
                                    vd[:, tt, :], ps[:], rsqh[:, tt:tt + 1],
                                    vd[:, tt, :], Alu.mult, Alu.add,
                                )
                        if h == 3 and not no_cc:
                            emit_ar(vd, layer, c)

                    def U(layer, half):
                        # v = ln(v + ln(vd)); one t-chunk at a time
                        vd = vds[layer]
                        tt0 = 4 * half
                        sl = slice(tt0, tt0 + 4)
                        s8 = spool.tile([P, 4], f32, tag="vs8", name=f"vs8_{rep}_{layer}_{half}")
                        q8 = spool.tile([P, 4], f32, tag="vq8", name=f"vq8_{rep}_{layer}_{half}")
                        for i in range(4):
                            tt = tt0 + i
                            nc.vector.tensor_reduce(s8[:, i:i + 1], vd[:, tt, :], axis=AX.X, op=Alu.add)
                            dummy = scrp.tile([P, D], bf16, tag="lndum", name="lndum")
                            nc.scalar.activation(dummy[:], vd[:, tt, :], Act.Square, accum_out=q8[:, i:i + 1])
                        mu8 = spool.tile([P, 4], f32, tag="vmu8", name=f"vmu8_{rep}_{layer}_{half}")
                        nc.vector.tensor_scalar(mu8[:], s8[:], 1.0 / D, None, Alu.mult)
                        mu28 = spool.tile([P, 4], f32, tag="vmu28", name=f"vmu28_{rep}_{layer}_{half}")
                        nc.vector.tensor_tensor(mu28[:], mu8[:], mu8[:], op=Alu.mult)
                        var8 = spool.tile([P, 4], f32, tag="vvar8", name=f"vvar8_{rep}_{layer}_{half}")
                        nc.vector.scalar_tensor_tensor(
                            var8[:], q8[:], 1.0 / D, mu28[:], Alu.mult, Alu.subtract
                        )
                        stdv = spool.tile([P, 4], f32, tag="vstd8", name=f"vstd8_{rep}_{layer}_{half}")
                        nc.scalar.activation(stdv[:], var8[:], Act.Sqrt, bias=epsb[:])
                        rsqv = spool.tile([P, 4], f32, tag="vrsq8", name=f"vrsq8_{rep}_{layer}_{half}")
                        nc.vector.reciprocal(rsqv[:], stdv[:])
                        # w = v + ln(vd); w rows are exactly zero-mean -> RMS only.
                        # per-tt chains so the first transposes start early
                        for i in range(4):
                            tt = tt0 + i
                            lnvd = scrp.tile([P, D], bf16, tag="lnvd", name="lnvd")
                            nc.vector.tensor_scalar(
                                lnvd[:], vd[:, tt, :], mu8[:, i:i + 1], rsqv[:, i:i + 1],
                                Alu.subtract, Alu.mult
                            )
                            nc.vector.tensor_tensor(vbf[:, tt, :], vbf[:, tt, :], lnvd[:], op=Alu.add)
                            q2 = spool.tile([P, 1], f32, tag="q2")
                            dummy2 = scrp.tile([P, D], bf16, tag="lndum", name="lndum2")
                            nc.scalar.activation(dummy2[:], vbf[:, tt, :], Act.Square, accum_out=q2[:])
                            std2 = spool.tile([P, 1], f32, tag="std2")
                            nc.scalar.activation(std2[:], q2[:], Act.Sqrt, bias=epsb[:], scale=1.0 / D)
                            rsq2 = spool.tile([P, 1], f32, tag="rsq2")
                            nc.vector.reciprocal(rsq2[:], std2[:])
                            nc.vector.tensor_scalar(
                                vbf[:, tt, :], vbf[:, tt, :], rsq2[:], None, Alu.mult
                            )
                            transpose_tt(tt)

                    # lm head pieces
                    CG = 500
                    NCG = VLOC // CG
                    lmh_tiles = {}

                    def load_cg(cg, pool=None):
                        tags = {id(slab): "slab", id(wxp): "wx", id(wyp): "wy"}
                        rhs = (pool or lmhp).tile(
                            [P, 4, CG], bf16,
                            tag=tags.get(id(pool), "lmh"),
                            name=f"lmh_{rep}_{cg}",
                        )
                        nc.sync.dma_start(rhs[:], LMH[:, :, cg * CG:(cg + 1) * CG])
                        lmh_tiles[cg] = rhs

                    def lm_cg(cg, tts):
                        # tt 4-7 groups ride the xp1 tag: they wait on the
                        # final v chunk-B and must not block the mm ring.
                        # out DMAs batched per tt-pair and alternated between
                        # the SP and ACT queues to stay under the PE rate.
                        rhs = lmh_tiles[cg]
                        for tp0 in range(tts.start, tts.stop, 2):
                            ob = obp.tile([P, 2, CG], bf16, tag="ob", name=f"ob_{rep}_{cg}_{tp0}")
                            for j in range(2):
                                tt = tp0 + j
                                ps = mmps.tile(
                                    [P, 512], f32, tag="mm" if tt < 4 else "xp1",
                                    bufs=3 if tt < 4 else 2, name="lm_ps",
                                )
                                for ds in range(4):
                                    nc.tensor.matmul(
                                        ps[:, :CG],
                                        vT[:, ds, tt * 128:(tt + 1) * 128],
                                        rhs[:, ds, :],
                                        start=(ds == 0),
                                        stop=(ds == 3),
                                    )
                                if (tt + j) % 2 == 0:
                                    nc.vector.tensor_copy(ob[:, j, :], ps[:, :CG])
                                else:
                                    nc.scalar.copy(ob[:, j, :], ps[:, :CG])
                            q = nc.sync if (cg + tp0 // 2) % 2 == 0 else nc.scalar
                            q.dma_start(
                                OUT[tp0 * 128:(tp0 + 2) * 128, cg * CG:(cg + 1) * CG]
                                .rearrange("(o p) c -> p o c", p=P),
                                ob[:],
                            )

                    # ---- the pipeline ----
                    get_vd(0)
                    X(0, 0, 0, strips=True); X(0, 1, 0); X(0, 0, 1); X(0, 1, 1)
                    S(0, 0, 0); S(0, 0, 1)
                    for layer in range(NL):
                        if layer == NL - 1:
                            load_cg(0); load_cg(1)
                        X(layer, 2, 0); X(layer, 2, 1)
                        S(layer, 1, 0); S(layer, 1, 1)
                        X(layer, 3, 0); X(layer, 3, 1)
                        S(layer, 2, 0)
                        S(layer, 2, 1, part="front")
                        S(layer, 3, 0)
                        S(layer, 2, 1, part="back")
                        S(layer, 3, 1)
                        if layer < NL - 1:
                            nxt = layer + 1
                            get_vd(nxt)
                            U(layer, 0)
                            X(nxt, 0, 0, strips=True); X(nxt, 1, 0)
                            S(nxt, 0, 0)
                            U(layer, 1)
                            X(nxt, 0, 1); X(nxt, 1, 1)
                            S(nxt, 0, 1)
                        else:
                            # extra boundary rhs tiles ride dead slab/wx/wy slots
                            for cg in range(2, 6):
                                load_cg(cg, pool=slab)
                            for cg in (6, 7):
                                load_cg(cg, pool=wxp)
                            for cg in (8, 9):
                                load_cg(cg, pool=wyp)
                            U(layer, 0)
                            for cg in range(10):
                                lm_cg(cg, range(4))
                            U(layer, 1)
                            for cg in range(10):
                                lm_cg(cg, range(4, 8))
                            load_cg(10)
                            for cg in range(10, NCG):
                                if cg + 1 < NCG:
                                    load_cg(cg + 1)
                                lm_cg(cg, range(8))

    nc.finalize()
    return nc


# host-side input prep -------------------------------------------------------

_PERM = np.concatenate([np.arange(0, N, 2), np.arange(1, N, 2)])  # evens then odds


def prep_in_maps(idx, embed, decoder_x, decoder_y, encoder, lm_head):
    idx = np.asarray(idx).astype(np.int32)
    embed = np.ascontiguousarray(np.asarray(embed, dtype=np.float32))
    decoder_x = np.asarray(decoder_x, dtype=np.float32)
    decoder_y = np.asarray(decoder_y, dtype=np.float32)
    encoder = np.asarray(encoder, dtype=np.float32)
    lm_head = np.asarray(lm_head, dtype=np.float32)

    bf = ml_dtypes.bfloat16

    # rope tables, [pair-idx striped over (p, s), t]
    i = np.arange(N // 2, dtype=np.float64)
    freqs = 1.0 / (10000.0 ** (2.0 * i / N))          # (512,)
    ang = np.arange(T, dtype=np.float64)[None, :] * freqs[:, None]  # (512, T)
    cos_t = np.cos(ang).reshape(4, P, T).transpose(1, 0, 2).astype(bf)
    sin_t = np.sin(ang).reshape(4, P, T).transpose(1, 0, 2).astype(bf)

    # strict-causal diag masks: mask[p, k, j] = 1 if (128k + p) < j else 0
    pp = np.arange(P)[:, None]
    jj = np.arange(512)[None, :]
    mask = (pp < jj).astype(ml_dtypes.bfloat16)

    in_maps = []
    for c in range(N_CORES):
        b, tp = c // 2, c % 2
        hs = slice(tp * NHL, tp * NHL + NHL)

        idx_b = idx[b]                                     # (T,)
        idx_t = idx_b.reshape(T // P, P).T.copy()          # [P, 8]

        wx = decoder_x[hs][:, :, _PERM]                    # (4, D, N)
        wy = decoder_y[hs][:, :, _PERM]
        # -> [P, h, nt, ds, j]: w[h, ds*128+p, nt*128+j]
        def wlay(w):
            w = w.reshape(NHL, 4, P, 8, P)                 # (h, ds, p, nt, j)
            return np.ascontiguousarray(w.transpose(2, 0, 3, 1, 4)).astype(bf)

        enc = encoder.reshape(NH, N, D)[hs][:, _PERM, :]   # (4, N, D)
        enc = enc.reshape(NHL, 8, P, D)                    # (h, ns, p, d)
        enc_t = np.ascontiguousarray(enc.transpose(2, 0, 1, 3)).astype(bf)

        lmh = lm_head[:, tp * VLOC:(tp + 1) * VLOC]        # (D, VLOC)
        lmh = lmh.reshape(4, P, VLOC)                      # (ds, p, u)
        lmh_t = np.ascontiguousarray(lmh.transpose(1, 0, 2)).astype(bf)

        in_maps.append({
            "emb": embed,
            "idx": np.ascontiguousarray(idx_t),
            "wx": wlay(wx),
            "wy": wlay(wy),
            "enc": enc_t,
            "lmh": lmh_t,
            "cos": np.ascontiguousarray(cos_t),
            "sin": np.ascontiguousarray(sin_t),
            "mask": np.ascontiguousarray(mask),
        })
    return in_maps


_NC_CACHE = {}


def get_nc():
    if "nc" not in _NC_CACHE:
        _NC_CACHE["nc"] = build_nc()
    return _NC_CACHE["nc"]


def kernel(idx, embed, decoder_x, decoder_y, encoder, lm_head):
    idx = np.asarray(idx)
    in_maps = prep_in_maps(idx, embed, decoder_x, decoder_y, encoder, lm_head)
    nc = get_nc()
    res = run_bass_kernel_spmd(nc, in_maps, core_ids=list(range(N_CORES)))
    logits = np.empty((B, T, VOCAB), dtype=np.float32)
    for c in range(N_CORES):
        b, tp = c // 2, c % 2
        logits[b, :, tp * VLOC:(tp + 1) * VLOC] = res.results[c]["out"]
    return logits

